# revision 14
# baseline (speedup 1.0000x reference)
"""Trainium2 Bass kernel for nn_EnsembleWorldModel (E=8 ensemble of residual
MLP world models), sharded one ensemble member per NeuronCore across 8 cores.

Layout: activations live as [hidden on partitions, tokens on free axis], so
every matmul uses the natural [in, out] weight layout as the stationary
operand and no transposes are needed anywhere.  LayerNorm reduces over the
partition axis, done with PE side-chains (sum(z) via host-precomputed
column-summed weights, sum(z^2) via a ones-vector matmul over ACT-squared z);
per-token scale/shift are broadcast across partitions with K=1 matmuls.  The
ensemble mean/var reduction is an on-device AllToAll of the per-member head
outputs followed by a local fp32 reduction of each core's token block.
"""

import os
import sys

for _p in ("/opt/trn_rl_repo", "/opt/pypackages"):
    if os.path.isdir(_p) and _p not in sys.path:
        sys.path.append(_p)

import numpy as np
import ml_dtypes

BF16 = ml_dtypes.bfloat16

# Problem constants (hardcoded per the contract).
E = 8
S = 512
A = 64
H = 2048
L = 3
B = 8192
LN_EPS = 1e-5
KIN = S + A            # 576
HT = H // 128          # 16 hidden partition-tiles

# Tunables.
TC = 512               # token chunk (matmul moving free dim)
N_CORES = 8

_RUNNER_CACHE = {}


def _kt_sizes(k):
    out = []
    while k > 0:
        out.append(min(128, k))
        k -= 128
    return out


def build_nc(n_cores=N_CORES, tokens=B, tc=TC, with_collective=True,
             compile_=True):
    """Build + schedule the SPMD program (identical on every core)."""
    import contextlib
    import concourse.bacc as bacc
    import concourse.mybir as mybir
    import concourse.tile as tile

    f32 = mybir.dt.float32
    bf16 = mybir.dt.bfloat16
    AF = mybir.ActivationFunctionType
    OP = mybir.AluOpType
    AX = mybir.AxisListType

    blk = tokens // n_cores          # tokens handled per core post-exchange
    nch = tokens // tc               # token chunks
    assert blk % tc == 0 or tc % blk == 0
    prows = S + 2                    # rows in the exchange tensor (d, rew, done)

    nc = bacc.Bacc("TRN2", target_bir_lowering=False, debug=False,
                   num_devices=n_cores)

    # ---- DRAM I/O ----------------------------------------------------------
    xT = nc.dram_tensor("xT", [KIN, tokens], bf16, kind="ExternalInput").ap()
    w_in = nc.dram_tensor("w_in", [KIN, H], bf16, kind="ExternalInput").ap()
    w_h = nc.dram_tensor("w_h", [L, H, H], bf16, kind="ExternalInput").ap()
    w_st = nc.dram_tensor("w_st", [H, S], bf16, kind="ExternalInput").ap()
    w_rd = nc.dram_tensor("w_rd", [H, 2], bf16, kind="ExternalInput").ap()
    NCB = 4 * 48 + 4 + 8
    cb = nc.dram_tensor("cb", [128, NCB], f32, kind="ExternalInput").ap()
    NWB = 5 + L * 16
    wb = nc.dram_tensor("wb", [128, NWB], bf16, kind="ExternalInput").ap()
    sblk = nc.dram_tensor("sblk", [S, blk], f32, kind="ExternalInput").ap()

    out_ms = nc.dram_tensor("out_ms", [S, blk], f32, kind="ExternalOutput").ap()
    out_u = nc.dram_tensor("out_u", [1, blk], f32, kind="ExternalOutput").ap()
    out_rd = nc.dram_tensor("out_rd", [2, blk], f32, kind="ExternalOutput").ap()

    pa = nc.dram_tensor("pa_stage", [n_cores, prows, blk], bf16).ap()
    qa = nc.dram_tensor("qa_stage", [n_cores, prows, blk], bf16).ap()

    def bcol(li, ht):
        return li * 48 + ht

    def gcol(li, ht):
        return li * 48 + 16 + ht

    def ecol(li, ht):
        return li * 48 + 32 + ht

    CST = 4 * 48
    CRW = CST + 4
    CDN = CST + 5
    CBS = CST + 6

    lays = [dict(kts=_kt_sizes(KIN),
                 wdram=lambda kt, cl: w_in[kt * 128:kt * 128 + min(128, KIN - kt * 128), cl],
                 wbc=0, li=0)]
    for l in range(L):
        lays.append(dict(
            kts=_kt_sizes(H),
            wdram=(lambda l: (lambda kt, cl: w_h[l, kt * 128:(kt + 1) * 128, cl]))(l),
            wbc=5 + l * 16, li=1 + l))

    with tile.TileContext(nc) as tcx:
        with contextlib.ExitStack() as stack:
            ep = stack.enter_context

            cpool = ep(tcx.tile_pool(name="consts", bufs=1))
            wh_pool = ep(tcx.tile_pool(name="wh", bufs=34))
            wst_pool = ep(tcx.tile_pool(name="wst", bufs=16))
            wrd_pool = ep(tcx.tile_pool(name="wrd", bufs=16))
            xp = ep(tcx.tile_pool(name="xp", bufs=2))
            hp = ep(tcx.tile_pool(name="hp", bufs=4))
            zp = ep(tcx.tile_pool(name="zp", bufs=2))
            z2p = ep(tcx.tile_pool(name="z2p", bufs=3))
            znp = ep(tcx.tile_pool(name="znp", bufs=4))
            bcp = ep(tcx.tile_pool(name="bcp", bufs=4))
            statp = ep(tcx.tile_pool(name="statp", bufs=6))
            douts = ep(tcx.tile_pool(name="douts", bufs=4))
            zps = ep(tcx.tile_pool(name="zps", bufs=3, space="PSUM"))
            stps = ep(tcx.tile_pool(name="stps", bufs=2, space="PSUM"))
            bps = ep(tcx.tile_pool(name="bps", bufs=3, space="PSUM"))

            cb_t = cpool.tile([128, NCB], f32)
            nc.sync.dma_start(cb_t[:, :], cb[:, :])
            wb_t = cpool.tile([128, NWB], bf16)
            nc.sync.dma_start(wb_t[:, :], wb[:, :])
            ones_t = cpool.tile([128, 1], bf16)
            nc.vector.memset(ones_t[:, :], 1.0)
            onesrow = cpool.tile([1, 128], bf16)
            nc.vector.memset(onesrow[:, :], 1.0)
            negrow = cpool.tile([1, 128], bf16)
            nc.vector.memset(negrow[:, :], -1.0)

            def s1_chain(lay, rhs, st66):
                kts = lay["kts"]
                for kt, kk in enumerate(kts):
                    nc.tensor.matmul(
                        st66[0:1, :],
                        wb_t[:kk, lay["wbc"] + kt:lay["wbc"] + kt + 1],
                        rhs[kt], start=(kt == 0), stop=(kt == len(kts) - 1))

            def quarter_mms(lay, wtsq, q, rhs, z_sb, st66, li):
                kts = lay["kts"]
                nkt = len(kts)
                for ht4 in range(4):
                    ht = q * 4 + ht4
                    zt = zps.tile([128, tc], f32, tag="zps")
                    for kt, kk in enumerate(kts):
                        nc.tensor.matmul(
                            zt[:, :],
                            wtsq[kt][:kk, ht4 * 128:(ht4 + 1) * 128],
                            rhs[kt], start=(kt == 0), stop=(kt == nkt - 1))
                    bias_ap = cb_t[:, bcol(li, ht):bcol(li, ht) + 1]
                    nc.scalar.activation(z_sb[:, ht * tc:(ht + 1) * tc],
                                         zt[:, :], AF.Identity, bias=bias_ap)
                    z2t = z2p.tile([128, tc], bf16, tag="z2")
                    nc.scalar.activation(z2t[:, :], zt[:, :], AF.Square,
                                         bias=bias_ap)
                    nc.tensor.matmul(st66[32:33, :], ones_t[:, 0:1], z2t[:, :],
                                     start=(ht == 0), stop=(ht == HT - 1))

            def stats_norm(li, z_sb, hprev, st66):
                """LN stats + normalize + gelu + residual.  Returns h_new."""
                def srow(nm):
                    return statp.tile([1, tc], f32, tag="srow", name=nm)
                mu, musq, museps, veps, sd, r0, rr, vrr, u15, rfin, cmu = (
                    srow(n) for n in ("mu", "musq", "museps", "veps", "sd",
                                      "r0", "rr", "vrr", "u15", "rfin", "cmu"))
                nc.vector.tensor_scalar(mu, st66[0:1, :],
                                        cb_t[0:1, CBS + li:CBS + li + 1],
                                        1.0 / H, OP.add, OP.mult)
                nc.vector.tensor_tensor(musq, mu, mu, OP.mult)
                nc.vector.tensor_scalar(museps, musq, LN_EPS, None, OP.subtract)
                nc.vector.scalar_tensor_tensor(veps, st66[32:33, :], 1.0 / H,
                                               museps, OP.mult, OP.subtract)
                nc.scalar.activation(sd, veps, AF.Sqrt)
                nc.vector.reciprocal(r0, sd)
                nc.vector.tensor_tensor(rr, r0, r0, OP.mult)
                nc.vector.tensor_tensor(vrr, veps, rr, OP.mult)
                nc.vector.tensor_scalar(u15, vrr, -0.5, 1.5, OP.mult, OP.add)
                nc.vector.tensor_tensor(rfin, r0, u15, OP.mult)
                nc.vector.tensor_tensor(cmu, mu, rfin, OP.mult)
                a_bf = bcp.tile([1, tc], bf16, tag="abf")
                nc.vector.tensor_copy(a_bf[:, :], rfin)
                c_bf = bcp.tile([1, tc], bf16, tag="abf")
                nc.vector.tensor_copy(c_bf[:, :], cmu)
                rb_ps = bps.tile([128, tc], f32, tag="bps")
                nc.tensor.matmul(rb_ps[:, :], onesrow[0:1, :], a_bf[:, :])
                cb_ps = bps.tile([128, tc], f32, tag="bps")
                nc.tensor.matmul(cb_ps[:, :], negrow[0:1, :], c_bf[:, :])
                rb = bcp.tile([128, tc], bf16, tag="bbig")
                nc.scalar.copy(rb[:, :], rb_ps[:, :])
                cbb = bcp.tile([128, tc], bf16, tag="bbig")
                nc.scalar.copy(cbb[:, :], cb_ps[:, :])
                h_new = hp.tile([128, HT * tc], bf16, tag="h")
                for ht in range(HT):
                    sl = slice(ht * tc, (ht + 1) * tc)
                    zn = znp.tile([128, tc], bf16, tag="zn")
                    nc.vector.tensor_tensor(zn[:, :], z_sb[:, sl], rb[:, :],
                                            OP.mult)
                    zn2 = znp.tile([128, tc], bf16, tag="zn")
                    nc.vector.tensor_tensor(zn2[:, :], zn[:, :], cbb[:, :],
                                            OP.add)
                    g_ap = cb_t[:, gcol(li, ht):gcol(li, ht) + 1]
                    e_ap = cb_t[:, ecol(li, ht):ecol(li, ht) + 1]
                    if hprev is None:
                        nc.scalar.activation(h_new[:, sl], zn2[:, :], AF.Gelu,
                                             bias=e_ap, scale=g_ap)
                    else:
                        zg = znp.tile([128, tc], bf16, tag="zn")
                        nc.scalar.activation(zg[:, :], zn2[:, :], AF.Gelu,
                                             bias=e_ap, scale=g_ap)
                        nc.vector.tensor_tensor(h_new[:, sl], zg[:, :],
                                                hprev[:, sl], OP.add)
                return h_new

            def head_body(c, rhs, wsts, wrds, st66):
                blki, off = (c * tc) // blk, (c * tc) % blk
                for stt in range(4):
                    dps = zps.tile([128, tc], f32, tag="zps")
                    for kt in range(HT):
                        nc.tensor.matmul(
                            dps[:, :], wsts[kt][:, stt * 128:(stt + 1) * 128],
                            rhs[kt], start=(kt == 0), stop=(kt == HT - 1))
                    dbf = douts.tile([128, tc], bf16, tag="dbf")
                    nc.scalar.activation(dbf[:, :], dps[:, :], AF.Identity,
                                         bias=cb_t[:, CST + stt:CST + stt + 1])
                    nc.sync.dma_start(
                        pa[blki, stt * 128:(stt + 1) * 128, off:off + tc],
                        dbf[:, :])
                for kt in range(HT):
                    nc.tensor.matmul(st66[64:65, :], wrds[kt][:, 0:1], rhs[kt],
                                     start=(kt == 0), stop=(kt == HT - 1))
                for kt in range(HT):
                    nc.tensor.matmul(st66[0:1, :], wrds[kt][:, 1:2], rhs[kt],
                                     start=(kt == 0), stop=(kt == HT - 1))
                rwt = douts.tile([1, tc], bf16, tag="rrow")
                nc.scalar.activation(rwt[:, :], st66[64:65, :], AF.Identity,
                                     bias=cb_t[0:1, CRW:CRW + 1])
                nc.sync.dma_start(pa[blki, S:S + 1, off:off + tc], rwt[:, :])
                dnt = douts.tile([1, tc], bf16, tag="rrow")
                nc.scalar.activation(dnt[:, :], st66[0:1, :], AF.Sigmoid,
                                     bias=cb_t[0:1, CDN:CDN + 1])
                nc.sync.dma_start(pa[blki, S + 1:S + 2, off:off + tc], dnt[:, :])

            # ---- main loop over chunk pairs --------------------------------
            step = 2 if nch >= 2 else 1
            for cp in range(0, nch, step):
                cs = list(range(cp, cp + step))
                rhss = {}
                hprevs = {}
                for c in cs:
                    xt = xp.tile([128, 5 * tc], bf16, tag="x")
                    for kt, kk in enumerate(_kt_sizes(KIN)):
                        nc.sync.dma_start(
                            xt[:kk, kt * tc:kt * tc + tc],
                            xT[kt * 128:kt * 128 + kk, c * tc:(c + 1) * tc])
                    rhss[c] = [xt[:kk, kt * tc:(kt + 1) * tc]
                               for kt, kk in enumerate(_kt_sizes(KIN))]
                    hprevs[c] = None
                last_st = {}
                for lay in lays:
                    li = lay["li"]
                    sts = {}
                    zsbs = {}
                    for c in cs:
                        sts[c] = stps.tile([66, tc], f32, tag="st", name=f"st_{cp}_{lay['li']}_{c}")
                        s1_chain(lay, rhss[c], sts[c])
                        zsbs[c] = zp.tile([128, HT * tc], bf16, tag="z", name=f"zsb_{cp}_{lay['li']}_{c}")
                    for q in range(4):
                        wtsq = []
                        for kt, kk in enumerate(lay["kts"]):
                            t = wh_pool.tile([128, 512], bf16, tag="wh")
                            nc.sync.dma_start(
                                t[:kk, :],
                                lay["wdram"](kt, slice(q * 512, (q + 1) * 512)))
                            wtsq.append(t)
                        for c in cs:
                            quarter_mms(lay, wtsq, q, rhss[c], zsbs[c],
                                        sts[c], li)
                    for c in cs:
                        h_new = stats_norm(li, zsbs[c], hprevs[c], sts[c])
                        hprevs[c] = h_new
                        rhss[c] = [h_new[:, kt * tc:(kt + 1) * tc]
                                   for kt in range(HT)]
                        last_st[c] = sts[c]
                # heads
                wsts = []
                for kt in range(HT):
                    t = wst_pool.tile([128, S], bf16, tag="wst")
                    nc.sync.dma_start(t[:, :], w_st[kt * 128:(kt + 1) * 128, :])
                    wsts.append(t)
                wrds = []
                for kt in range(HT):
                    t = wrd_pool.tile([128, 2], bf16, tag="wrd")
                    nc.sync.dma_start(t[:, :], w_rd[kt * 128:(kt + 1) * 128, :])
                    wrds.append(t)
                for c in cs:
                    head_body(c, rhss[c], wsts, wrds, last_st[c])

            if with_collective:
                nc.gpsimd.collective_compute(
                    "AllToAll", OP.bypass,
                    replica_groups=[list(range(n_cores))],
                    ins=[pa.opt()], outs=[qa.opt()],
                )
                qsrc = qa
            else:
                qsrc = pa

        # ---- finalize: local fp32 reduction over ensemble members ----------
        with contextlib.ExitStack() as stack2:
            ep2 = stack2.enter_context
            fq = ep2(tcx.tile_pool(name="fq", bufs=2))
            fsq = ep2(tcx.tile_pool(name="fsq", bufs=1))
            facc = ep2(tcx.tile_pool(name="facc", bufs=6))
            fsb = ep2(tcx.tile_pool(name="fsb", bufs=1))
            fone = ep2(tcx.tile_pool(name="fone", bufs=1))
            ups = ep2(tcx.tile_pool(name="ups", bufs=2, space="PSUM"))

            u_segs = [(s, min(s + 512, blk)) for s in range(0, blk, 512)]
            onesf = fone.tile([128, 1], f32)
            nc.vector.memset(onesf[:, :], 1.0)
            sbt = fsb.tile([128, 4 * blk], f32)
            for stt in range(4):
                nc.sync.dma_start(sbt[:, stt * blk:(stt + 1) * blk],
                                  sblk[stt * 128:(stt + 1) * 128, :])
            qv = qsrc.rearrange("e r t -> r e t")
            u_ps = [ups.tile([1, b - a], f32, tag="ups", name=f"ups_{a}") for a, b in u_segs]
            for stt in range(4):
                qt = fq.tile([128, n_cores * blk], bf16, tag="fq")
                nc.sync.dma_start(qt[:, :],
                                  qv[stt * 128:(stt + 1) * 128, :, :])
                qt3 = qt[:, :].rearrange("p (e t) -> p t e", e=n_cores)
                s1 = facc.tile([128, blk], f32, tag="facc")
                nc.vector.tensor_reduce(s1[:, :], qt3, axis=AX.X, op=OP.add)
                sq = fsq.tile([128, n_cores * blk], f32, tag="fsq")
                nc.scalar.activation(sq[:, :], qt[:, :], AF.Square)
                sq3 = sq[:, :].rearrange("p (e t) -> p t e", e=n_cores)
                s2 = facc.tile([128, blk], f32, tag="facc")
                nc.vector.tensor_reduce(s2[:, :], sq3, axis=AX.X, op=OP.add)
                ms = facc.tile([128, blk], f32, tag="facc")
                nc.vector.scalar_tensor_tensor(
                    ms[:, :], s1[:, :], 1.0 / n_cores,
                    sbt[:, stt * blk:(stt + 1) * blk], OP.mult, OP.add)
                nc.sync.dma_start(out_ms[stt * 128:(stt + 1) * 128, :], ms[:, :])
                s1sq = facc.tile([128, blk], f32, tag="facc")
                nc.scalar.activation(s1sq[:, :], s1[:, :], AF.Square,
                                     scale=float(n_cores) ** -0.5)
                var7 = facc.tile([128, blk], f32, tag="facc")
                nc.vector.tensor_tensor(var7[:, :], s2[:, :], s1sq[:, :],
                                        OP.subtract)
                for ui, (a, b) in enumerate(u_segs):
                    nc.tensor.matmul(u_ps[ui][:, :], onesf[:, 0:1],
                                     var7[:, a:b],
                                     start=(stt == 0), stop=(stt == 3))
            urow = facc.tile([1, blk], f32, tag="urow", bufs=1)
            for ui, (a, b) in enumerate(u_segs):
                nc.vector.tensor_scalar(urow[:, a:b], u_ps[ui][:, :],
                                        1.0 / (S * (n_cores - 1)), None,
                                        OP.mult)
            nc.sync.dma_start(out_u[:, :], urow[:, :])
            qrd = fq.tile([2, n_cores * blk], bf16, tag="frd", bufs=1)
            nc.sync.dma_start(qrd[:, :], qv[S:S + 2, :, :])
            qrd3 = qrd[:, :].rearrange("p (e t) -> p t e", e=n_cores)
            rds = facc.tile([2, blk], f32, tag="frd2", bufs=2)
            nc.vector.tensor_reduce(rds[:, :], qrd3, axis=AX.X, op=OP.add)
            rdm = facc.tile([2, blk], f32, tag="frd2", bufs=2)
            nc.vector.tensor_scalar(rdm[:, :], rds[:, :], 1.0 / n_cores, None,
                                    OP.mult)
            nc.sync.dma_start(out_rd[:, :], rdm[:, :])

    if compile_:
        nc.compile()
    return nc


def _host_prep(inputs, n_cores=N_CORES, tokens=B):
    """Build the per-core input maps (weight layout prep on host)."""
    state = np.asarray(inputs["state"], np.float32)[:tokens]
    action = np.asarray(inputs["action"], np.float32)[:tokens]
    blk = tokens // n_cores
    x = np.concatenate([state, action], axis=1)          # [B, 576]
    xT_bf = np.ascontiguousarray(x.T).astype(BF16)       # [576, B]
    stT = np.ascontiguousarray(state.T)                  # [512, B] fp32

    W_in = np.asarray(inputs["W_in"], np.float32)
    W_h = np.asarray(inputs["W_h"], np.float32)
    W_state = np.asarray(inputs["W_state"], np.float32)
    W_rew = np.asarray(inputs["W_rew"], np.float32)
    W_done = np.asarray(inputs["W_done"], np.float32)
    b_in = np.asarray(inputs["b_in"], np.float32)
    g_in = np.asarray(inputs["g_in"], np.float32)
    be_in = np.asarray(inputs["be_in"], np.float32)
    b_h = np.asarray(inputs["b_h"], np.float32)
    g_h = np.asarray(inputs["g_h"], np.float32)
    be_h = np.asarray(inputs["be_h"], np.float32)
    b_state = np.asarray(inputs["b_state"], np.float32)
    b_rew = np.asarray(inputs["b_rew"], np.float32)
    b_done = np.asarray(inputs["b_done"], np.float32)

    NCB = 4 * 48 + 4 + 8
    NWB = 5 + L * 16
    in_maps = []
    for e in range(n_cores):
        em = e % E
        cbm = np.zeros((128, NCB), np.float32)
        lays = [(b_in[em], g_in[em], be_in[em])] + [
            (b_h[em, l], g_h[em, l], be_h[em, l]) for l in range(L)]
        for li, (bb, gg, ee) in enumerate(lays):
            for ht in range(HT):
                cbm[:, li * 48 + ht] = bb[ht * 128:(ht + 1) * 128]
                cbm[:, li * 48 + 16 + ht] = gg[ht * 128:(ht + 1) * 128]
                cbm[:, li * 48 + 32 + ht] = ee[ht * 128:(ht + 1) * 128]
            cbm[0, 4 * 48 + 6 + li] = float(bb.sum())
        for stt in range(4):
            cbm[:, 4 * 48 + stt] = b_state[em, stt * 128:(stt + 1) * 128]
        cbm[0, 4 * 48 + 4] = float(b_rew[em, 0])
        cbm[0, 4 * 48 + 5] = float(b_done[em, 0])

        wbm = np.zeros((128, NWB), np.float32)
        wbar_in = W_in[em].sum(axis=1)
        for kt, kk in enumerate(_kt_sizes(KIN)):
            wbm[:kk, kt] = wbar_in[kt * 128:kt * 128 + kk]
        for l in range(L):
            wbar = W_h[em, l].sum(axis=1)
            for kt in range(16):
                wbm[:, 5 + l * 16 + kt] = wbar[kt * 128:(kt + 1) * 128]

        in_maps.append({
            "xT": xT_bf,
            "w_in": W_in[em].astype(BF16),
            "w_h": W_h[em].astype(BF16),
            "w_st": W_state[em].astype(BF16),
            "w_rd": np.concatenate([W_rew[em], W_done[em]], axis=1).astype(BF16),
            "cb": cbm,
            "wb": wbm.astype(BF16),
            "sblk": np.ascontiguousarray(stT[:, e * blk:(e + 1) * blk]),
        })
    return in_maps


def _postprocess(results, n_cores=N_CORES):
    ms = np.concatenate([r["out_ms"] for r in results], axis=1)   # [512, B]
    mean_state = np.ascontiguousarray(ms.T)                       # [B, 512]
    unc = np.concatenate([r["out_u"][0] for r in results])        # [B]
    rew = np.concatenate([r["out_rd"][0] for r in results])[:, None]
    don = np.concatenate([r["out_rd"][1] for r in results])[:, None]
    return (mean_state.astype(np.float32), rew.astype(np.float32),
            don.astype(np.float32), unc.astype(np.float32))


def _get_runner():
    key = "main"
    if key not in _RUNNER_CACHE:
        from concourse.bass_utils import run_bass_kernel_spmd
        nc = build_nc()

        def run(in_maps):
            res = run_bass_kernel_spmd(nc, in_maps,
                                       core_ids=list(range(N_CORES)))
            return res.results

        _RUNNER_CACHE[key] = (nc, run)
    return _RUNNER_CACHE[key]


def kernel(**inputs):
    nc, run = _get_runner()
    in_maps = _host_prep(inputs)
    results = run(in_maps)
    return _postprocess(results)
